# revision 1
# baseline (speedup 1.0000x reference)
"""Trainium2 Bass kernel for the rhyme soft-DP loss (CharLSTMLanguageModelPack).

loss[b] = softDP(sub[b]) + 10*(1 - p[b,0,tidx[b,0]])
  p = softmax(tail_logits, -1); sub[b,t,m] = sum_v p[b,t,v] * C[v, tidx[b,m]]
  softDP: dp[i,j] = softmin(dp[i-1,j]+10, dp[i,j-1]+10, dp[i-1,j-1]+sub[i-1,j-1])
  with softmin(a,b,c) = -log(e^-a + e^-b + e^-c)  (gamma=1)

Device strategy (pure data parallel over B, 1024 pairs/core):
  - Host sends logsumexp-normalized, transposed bf16 logits lT'[v, (b,t)]
    so that exp(lT') = softmax probabilities directly (no Z plumbing).
  - Host sends gathered phon-cost packs Cc[v, b*17+j]:
      j<16: C[:, tidx[b,j]],  j=16: onehot(tidx[b,0]) (first-char prob).
  - PE: per 8-pair tile, two 64-row "pack-4 all-pairs" matmuls per v-half:
      out[64=(4b,16t), 68=(4b',17j)] = pT^T @ Cc  accumulated over v.
  - DVE: blockmask multiply + segmented reduce over b' extracts the
    diagonal blocks -> G[b,t,j] (=sub numerators; j=16 -> first-char p).
  - exp-domain DP (softmin becomes LINEAR): E[i,j] = d*(E[i-1,j]+E[i,j-1])
    + S*E[i-1,j-1], S = exp(-sub), d = e^-10, via tensor_tensor_scan rows.
  - loss = -ln(E[16,16]) + 10 - 10*p_firstchar.
"""
import numpy as np
import ml_dtypes
from contextlib import ExitStack

import concourse.bass as bass
import concourse.tile as tile
from concourse import bacc, mybir
from concourse.bass_utils import run_bass_kernel_spmd

AP = bass.AP
FP32 = mybir.dt.float32
BF16 = mybir.dt.bfloat16

N_CORES = 8
B, T, M, V = 8192, 16, 16, 256
BSH = B // N_CORES            # 1024 pairs per core
NT = BSH // 8                 # 128 tiles of 8 pairs
BT = BSH * T                  # 16384 bt columns per core
J = 17                        # 16 sub cols + 1 first-char col
REG = 4 * J                   # 68 cols per pack-4 region
RPB = 7                       # regions per PSUM bank (7*68*4B = 1904 <= 2048)
INS_DEL = 10.0
D_COEF = float(np.exp(-INS_DEL))

_cache = {}

def _ap(t, off, dims):
    """Strided free-dim view of a tile: canonical partition dim + custom free dims."""
    base = t[:]
    return AP(base.tensor, base.offset + off, [list(base.ap[0])] + [list(d) for d in dims])



def _build_nc():
    nc = bacc.Bacc("TRN2", target_bir_lowering=False, debug=False,
                   num_devices=N_CORES)
    lt0 = nc.dram_tensor("lt0", [128, BT], mybir.dt.bfloat16, kind="ExternalInput")
    lt1 = nc.dram_tensor("lt1", [128, BT], mybir.dt.bfloat16, kind="ExternalInput")
    cc0 = nc.dram_tensor("cc0", [128, BSH * J], mybir.dt.bfloat16, kind="ExternalInput")
    cc1 = nc.dram_tensor("cc1", [128, BSH * J], mybir.dt.bfloat16, kind="ExternalInput")
    bmask = nc.dram_tensor("bmask", [128, RPB * REG], FP32, kind="ExternalInput")
    dmask = nc.dram_tensor("dmask", [128, 136], FP32, kind="ExternalInput")
    init0 = nc.dram_tensor("init0", [128, 136], FP32, kind="ExternalInput")
    ident = nc.dram_tensor("ident", [128, 128], mybir.dt.bfloat16, kind="ExternalInput")
    identf = nc.dram_tensor("identf", [128, 128], FP32, kind="ExternalInput")
    out = nc.dram_tensor("out", [128, 8], FP32, kind="ExternalOutput")

    with tile.TileContext(nc) as tc, ExitStack() as ctx:
        P = lambda name, bufs, **kw: ctx.enter_context(
            tc.tile_pool(name=name, bufs=bufs, **kw))
        const_pool = P("const", 1)
        pt_pool = P("pt", 1)
        cc_pool = P("cc", 3)
        ps_pool = P("ps", 4, space="PSUM")
        msk_pool = P("msk", 6)
        gall_pool = P("gall", 1)
        xp_pool = P("xp", 1)
        tp_pool = P("tp", 2, space="PSUM")
        d_pool = P("d", 1)
        e_pool = P("e", 3)
        fin_pool = P("fin", 1)

        # constants
        bm = const_pool.tile([128, RPB * REG], FP32, tag="bm", name="bm")
        nc.sync.dma_start(bm[:], bmask[:])
        dm = const_pool.tile([128, 136], FP32, tag="dm", name="dm")
        nc.sync.dma_start(dm[:], dmask[:])
        i0 = const_pool.tile([128, 136], FP32, tag="i0", name="i0")
        nc.sync.dma_start(i0[:], init0[:])
        idn = const_pool.tile([128, 128], mybir.dt.bfloat16, tag="idn", name="idn")
        nc.sync.dma_start(idn[:], ident[:])
        idnf = const_pool.tile([128, 128], FP32, tag="idnf", name="idnf")
        nc.sync.dma_start(idnf[:], identf[:])

        # probabilities pT[half][v=128, bt] (host sends softmax bf16 directly)
        pt = [pt_pool.tile([128, BT], mybir.dt.bfloat16, tag=f"pt{h}", name=f"pt{h}") for h in range(2)]
        lsrc = [lt0, lt1]
        ccs = [pt_pool.tile([128, BSH * J], mybir.dt.bfloat16, tag=f"cc{h}", name=f"cc{h}")
               for h in range(2)]
        ccsrc = [cc0, cc1]
        PT_CH = [2048] * 8
        CC_CH = [2176] * 8
        po = co = 0
        for k in range(len(PT_CH)):
            for h in range(2):
                nc.sync.dma_start(pt[h][:, po:po + PT_CH[k]],
                                  lsrc[h][:, po:po + PT_CH[k]])
                nc.sync.dma_start(ccs[h][:, co:co + CC_CH[k]],
                                  ccsrc[h][:, co:co + CC_CH[k]])
            po += PT_CH[k]; co += CC_CH[k]

        # G[b,t,j]: [128=(g,t), (c,j)] f32
        gall = gall_pool.tile([128, NT * J], FP32, tag="gall", name="gall")

        # matmul + extract, batches of RPB tiles
        c0 = 0
        while c0 < NT:
            nreg = min(RPB, NT - c0)
            ps = ps_pool.tile([128, 512], FP32, tag="ps", name="ps")
            for s in range(nreg):
                c = c0 + s
                for hh in range(2):       # partition half = 4-pair quad pack
                    for vh in range(2):   # contraction halves over v
                        nc.tensor.matmul(
                            ps[64 * hh:64 * hh + 64, REG * s:REG * s + REG],
                            pt[vh][:, c * 128 + 64 * hh: c * 128 + 64 * hh + 64],
                            ccs[vh][:, (c * 8 + 4 * hh) * J:
                                    (c * 8 + 4 * hh) * J + REG],
                            start=(vh == 0), stop=(vh == 1))
            mk = msk_pool.tile([128, RPB * REG], FP32, tag="mk", name="mk")
            nc.vector.tensor_tensor(
                _ap(mk, 0, [[REG, nreg], [1, 4], [4, J]]),
                _ap(ps, 0, [[REG, nreg], [J, 4], [1, J]]),
                _ap(bm, 0, [[REG, nreg], [J, 4], [1, J]]),
                mybir.AluOpType.mult)
            nc.vector.tensor_reduce(
                _ap(gall, c0 * J, [[J, nreg], [1, J]]),
                _ap(mk, 0, [[REG, nreg], [4, J], [1, 4]]),
                mybir.AxisListType.X, mybir.AluOpType.add)
            c0 += nreg

        # X' = exp(-G) over sub cols, written (m, c)-major bf16
        xp = xp_pool.tile([128, NT * 16], mybir.dt.bfloat16, tag="xp", name="xp")
        nc.scalar.activation(
            _ap(xp, 0, [[16, NT], [1, 16]]),
            _ap(gall, 0, [[J, NT], [1, 16]]),
            mybir.ActivationFunctionType.Exp, bias=0.0, scale=-1.0)
        # X2 = first-char probability col (j=16), f32
        x2 = xp_pool.tile([128, NT], FP32, tag="x2", name="x2")
        nc.vector.tensor_copy(
            _ap(x2, 0, [[1, NT]]),
            _ap(gall, 16, [[J, NT]]))

        # S rearrange: 16 transposes -> D[c, (i,g,j)] f32; +1 for first-char
        dt_ = d_pool.tile([128, 16 * 128], FP32, tag="dt", name="dt")
        for half in range(2):
            tp = tp_pool.tile([128, 1024], mybir.dt.bfloat16, tag="tp", name="tp")
            for mm_ in range(8):
                m = half * 8 + mm_
                nc.tensor.transpose(
                    tp[:, mm_ * 128:(mm_ + 1) * 128],
                    _ap(xp, m, [[16, 128]]), idn[:])
            nc.vector.tensor_copy(
                _ap(dt_, half * 8, [[1, 8], [16, 8], [128, 16]]),
                _ap(tp, 0, [[128, 8], [16, 8], [1, 16]]))
        tpf = tp_pool.tile([128, 128], FP32, tag="tpf", name="tpf", bufs=1)
        nc.tensor.transpose(tpf[:], x2[:], idnf[:])
        fct = fin_pool.tile([128, 8], FP32, tag="fct", name="fct")
        nc.vector.tensor_copy(
            _ap(fct, 0, [[1, 8]]),
            _ap(tpf, 0, [[16, 8]]))

        # DP in exp domain.  E tiles [128, (g8, jj17)]
        zt = e_pool.tile([128, 136], FP32, tag="tmp", name="tmp")
        nc.vector.memset(zt[:], 0.0)
        e_prev = e_pool.tile([128, 136], FP32, tag="e", name="e")
        nc.vector.tensor_tensor_scan(e_prev[:], dm[:], i0[:], 0.0,
                                     mybir.AluOpType.mult, mybir.AluOpType.add)
        a_t = e_pool.tile([128, 136], FP32, tag="a", name="a")
        for i in range(T):
            nc.vector.tensor_tensor(
                _ap(zt, 1, [[17, 8], [1, 16]]),
                _ap(dt_, i * 128, [[16, 8], [1, 16]]),
                _ap(e_prev, 0, [[17, 8], [1, 16]]),
                mybir.AluOpType.mult)
            nc.vector.scalar_tensor_tensor(
                a_t[:], e_prev[:], D_COEF, zt[:],
                mybir.AluOpType.mult, mybir.AluOpType.add)
            e_new = e_pool.tile([128, 136], FP32, tag="e", name="e")
            nc.vector.tensor_tensor_scan(e_new[:], dm[:], a_t[:], 0.0,
                                         mybir.AluOpType.mult, mybir.AluOpType.add)
            e_prev = e_new

        # loss = -ln(E[16,16]) + 10 - 10*fc
        lne = fin_pool.tile([128, 8], FP32, tag="lne", name="lne")
        nc.scalar.activation(
            lne[:],
            _ap(e_prev, 16, [[17, 8]]),
            mybir.ActivationFunctionType.Ln, bias=0.0, scale=1.0)
        t1 = fin_pool.tile([128, 8], FP32, tag="t1", name="t1")
        nc.vector.tensor_scalar(t1[:], fct[:], -10.0, 10.0,
                                mybir.AluOpType.mult, mybir.AluOpType.add)
        res = fin_pool.tile([128, 8], FP32, tag="res", name="res")
        nc.vector.tensor_tensor(res[:], t1[:], lne[:], mybir.AluOpType.subtract)
        nc.sync.dma_start(out[:], res[:])

    nc.finalize()
    return nc


def _host_prep(tail_logits, target_idx, phon_cost):
    l = np.asarray(tail_logits, dtype=np.float32)
    tidx = np.asarray(target_idx)
    C = np.asarray(phon_cost, dtype=np.float32)

    lmax = l.max(axis=-1, keepdims=True)
    e = np.exp(l - lmax)
    ln = e / e.sum(axis=-1, keepdims=True)  # softmax probabilities

    # Cc pack: [V, B*17]; col b*17+j
    cc = np.empty((V, B * J), dtype=np.float32)
    cols = cc.reshape(V, B, J)
    cols[:, :, :16] = C[:, tidx].astype(np.float32)
    oh = np.zeros((V, B), dtype=np.float32)
    oh[tidx[:, 0], np.arange(B)] = 1.0
    cols[:, :, 16] = oh
    cc_bf = cc.astype(ml_dtypes.bfloat16)

    # masks
    bmask = np.zeros((128, RPB * REG), dtype=np.float32)
    for p in range(128):
        q = (p // 16) % 4
        for s in range(RPB):
            bmask[p, s * REG + q * J:s * REG + (q + 1) * J] = 1.0
    dmask = np.zeros((128, 136), dtype=np.float32)
    init0 = np.zeros((128, 136), dtype=np.float32)
    for g in range(8):
        dmask[:, g * 17 + 1:(g + 1) * 17] = D_COEF
        init0[:, g * 17] = 1.0
    ident = np.eye(128, dtype=np.float32).astype(ml_dtypes.bfloat16)

    in_maps = []
    for k in range(N_CORES):
        sl = slice(k * BSH, (k + 1) * BSH)
        lt = np.ascontiguousarray(
            ln[sl].transpose(2, 0, 1).reshape(V, BT)).astype(ml_dtypes.bfloat16)
        ccsh = cc_bf[:, k * BSH * J:(k + 1) * BSH * J]
        in_maps.append({
            "lt0": np.ascontiguousarray(lt[:128]),
            "lt1": np.ascontiguousarray(lt[128:]),
            "cc0": np.ascontiguousarray(ccsh[:128]),
            "cc1": np.ascontiguousarray(ccsh[128:]),
            "bmask": bmask, "dmask": dmask, "init0": init0, "ident": ident,
            "identf": np.eye(128, dtype=np.float32),
        })
    return in_maps


def kernel(tail_logits, target_idx, phon_cost):
    if "nc" not in _cache:
        _cache["nc"] = _build_nc()
    nc = _cache["nc"]
    in_maps = _host_prep(tail_logits, target_idx, phon_cost)
    res = run_bass_kernel_spmd(nc, in_maps, core_ids=list(range(N_CORES)))
    outs = [res.results[k]["out"].reshape(BSH) for k in range(N_CORES)]
    return np.concatenate(outs).astype(np.float32)



# revision 3
# speedup vs baseline: 1.1934x; 1.1934x over previous
"""Trainium2 Bass kernel for the rhyme soft-DP loss (CharLSTMLanguageModelPack).

loss[b] = softDP(sub[b]) + 10*(1 - p[b,0,tidx[b,0]])
  p = softmax(tail_logits, -1); sub[b,t,m] = sum_v p[b,t,v] * C[v, tidx[b,m]]
  softDP: dp[i,j] = softmin(dp[i-1,j]+10, dp[i,j-1]+10, dp[i-1,j-1]+sub[i-1,j-1])
  with softmin(a,b,c) = -log(e^-a + e^-b + e^-c)  (gamma=1)

Device strategy (pure data parallel over B, 1024 pairs/core):
  - Host sends softmax probabilities (x128) and gathered phon-cost packs
    (x128) as fp8-e4m3, transposed: lt[v,(b,t)], cc[v,(b,j)] j<16.
    fp8 halves HBM traffic vs bf16; the 128x scale keeps typical values
    in e4m3's normal range. First-char prob is sent exact (fp32 [128,8]).
  - PE: per 8-pair tile, two 64-row "pack-4 all-pairs" matmuls per
    v-half: ps[64=(4b,16t), 64=(4b',16j)] accumulated over v.
  - DVE: blockmask multiply (bf16 out) + segmented 2x-mode reduce over
    b' extracts diagonal blocks -> G[b,t,j] = 16384*sub.
  - ACT: exp(-G/16384) -> S (bf16), and PSUM->SBUF copies.
  - exp-domain DP (softmin becomes LINEAR): E[i,j] = d*(E[i-1,j]+E[i,j-1])
    + S*E[i-1,j-1], d = e^-10, via tensor_tensor_scan rows (DVE, fp32).
  - loss = -ln(E[16,16]) + 10 - 10*p_firstchar.
"""
import numpy as np
import ml_dtypes
from contextlib import ExitStack

import concourse.bass as bass
import concourse.tile as tile
from concourse import bacc, mybir
from concourse.bass_utils import run_bass_kernel_spmd

AP = bass.AP
FP32 = mybir.dt.float32
BF16 = mybir.dt.bfloat16
FP8 = mybir.dt.float8e4

N_CORES = 8
B, T, M, V = 8192, 16, 16, 256
BSH = B // N_CORES            # 1024 pairs per core
NT = BSH // 8                 # 128 tiles of 8 pairs
BT = BSH * T                  # 16384 bt columns per core
J = 16                        # sub cols per pair (first-char handled on host)
REG = 4 * J                   # 64 cols per pack-4 region
RPB = 8                       # regions per PSUM bank (8*64*4B = 2048)
NCH = 4                       # input DMA chunks per dram tensor
INS_DEL = 10.0
D_COEF = float(np.exp(-INS_DEL))
SC = 128.0                    # fp8 scale for both p and C
ISC2 = 1.0 / (SC * SC)

_cache = {}

def _ap(t, off, dims):
    """Strided free-dim view of a tile: canonical partition dim + custom free dims."""
    base = t[:]
    return AP(base.tensor, base.offset + off, [list(base.ap[0])] + [list(d) for d in dims])


def _build_nc():
    nc = bacc.Bacc("TRN2", target_bir_lowering=False, debug=False,
                   num_devices=N_CORES)
    lt0 = nc.dram_tensor("lt0", [128, BT], FP8, kind="ExternalInput")
    lt1 = nc.dram_tensor("lt1", [128, BT], FP8, kind="ExternalInput")
    cc0 = nc.dram_tensor("cc0", [128, BSH * J], FP8, kind="ExternalInput")
    cc1 = nc.dram_tensor("cc1", [128, BSH * J], FP8, kind="ExternalInput")
    bmask = nc.dram_tensor("bmask", [128, RPB * REG], FP32, kind="ExternalInput")
    dmask = nc.dram_tensor("dmask", [128, 136], FP32, kind="ExternalInput")
    init0 = nc.dram_tensor("init0", [128, 136], FP32, kind="ExternalInput")
    ident = nc.dram_tensor("ident", [128, 128], BF16, kind="ExternalInput")
    fcin = nc.dram_tensor("fcin", [128, 8], FP32, kind="ExternalInput")
    out = nc.dram_tensor("out", [128, 8], FP32, kind="ExternalOutput")

    with tile.TileContext(nc) as tc, ExitStack() as ctx:
        P = lambda name, bufs, **kw: ctx.enter_context(
            tc.tile_pool(name=name, bufs=bufs, **kw))
        const_pool = P("const", 1)
        pt_pool = P("pt", 1)
        ps_pool = P("ps", 4, space="PSUM")
        msk_pool = P("msk", 4)
        gall_pool = P("gall", 1)
        xp_pool = P("xp", 1)
        tp_pool = P("tp", 2, space="PSUM")
        d_pool = P("d", 1)
        e_pool = P("e", 3)
        fin_pool = P("fin", 1)

        # constants (sync queue, before the big input loads)
        bm = const_pool.tile([128, RPB * REG], FP32, tag="bm", name="bm")
        nc.sync.dma_start(bm[:], bmask[:])
        dm = const_pool.tile([128, 136], FP32, tag="dm", name="dm")
        nc.sync.dma_start(dm[:], dmask[:])
        i0 = const_pool.tile([128, 136], FP32, tag="i0", name="i0")
        nc.sync.dma_start(i0[:], init0[:])
        idn = const_pool.tile([128, 128], BF16, tag="idn", name="idn")
        nc.sync.dma_start(idn[:], ident[:])
        fct = const_pool.tile([128, 8], FP32, tag="fct", name="fct")
        nc.sync.dma_start(fct[:], fcin[:])

        # inputs: fp8 halves; lt*/cc* chunks interleaved, issued from two
        # HWDGE engines (sync + scalar) so issue overhead overlaps.
        pt = [pt_pool.tile([128, BT], FP8, tag=f"pt{h}", name=f"pt{h}")
              for h in range(2)]
        ccs = [pt_pool.tile([128, BSH * J], FP8, tag=f"cc{h}", name=f"cc{h}")
               for h in range(2)]
        lsrc = [lt0, lt1]
        ccsrc = [cc0, cc1]
        CH = BT // NCH
        for k in range(NCH):
            sl = slice(k * CH, (k + 1) * CH)
            nc.sync.dma_start(pt[0][:, sl], lsrc[0][:, sl])
            nc.scalar.dma_start(pt[1][:, sl], lsrc[1][:, sl])
            nc.sync.dma_start(ccs[0][:, sl], ccsrc[0][:, sl])
            nc.scalar.dma_start(ccs[1][:, sl], ccsrc[1][:, sl])

        # G[b,t,j]: [128=(g,t), (c,j)] bf16 (16384*sub)
        gall = gall_pool.tile([128, NT * J], BF16, tag="gall", name="gall")

        # matmul + extract, batches of RPB tiles
        with nc.allow_low_precision(reason="sum of 4 masked bf16 products"):
            for c0 in range(0, NT, RPB):
                ps = ps_pool.tile([128, 512], FP32, tag="ps", name="ps")
                for s in range(RPB):
                    c = c0 + s
                    for hh in range(2):       # partition half = 4-pair quad pack
                        for vh in range(2):   # contraction halves over v
                            nc.tensor.matmul(
                                ps[64 * hh:64 * hh + 64, REG * s:REG * s + REG],
                                pt[vh][:, c * 128 + 64 * hh: c * 128 + 64 * hh + 64],
                                ccs[vh][:, (c * 8 + 4 * hh) * J:
                                        (c * 8 + 4 * hh) * J + REG],
                                start=(vh == 0), stop=(vh == 1))
                mk = msk_pool.tile([128, RPB * REG], BF16, tag="mk", name="mk")
                nc.vector.tensor_tensor(
                    _ap(mk, 0, [[REG, RPB], [1, 4], [4, J]]),
                    _ap(ps, 0, [[REG, RPB], [J, 4], [1, J]]),
                    _ap(bm, 0, [[REG, RPB], [J, 4], [1, J]]),
                    mybir.AluOpType.mult)
                nc.vector.tensor_reduce(
                    _ap(gall, c0 * J, [[J, RPB], [1, J]]),
                    _ap(mk, 0, [[REG, RPB], [4, J], [1, 4]]),
                    mybir.AxisListType.X, mybir.AluOpType.add)

        # S = exp(-G/16384), bf16, same (c,j) layout (ACT engine)
        xp = xp_pool.tile([128, NT * J], BF16, tag="xp", name="xp")
        nc.scalar.activation(xp[:], gall[:],
                             mybir.ActivationFunctionType.Exp,
                             bias=0.0, scale=-ISC2)

        # S rearrange: 16 PE transposes -> D[c, (i,g,j)] f32 via ACT copies
        dt_ = d_pool.tile([128, 16 * 128], FP32, tag="dt", name="dt")
        for half in range(2):
            tp = tp_pool.tile([128, 1024], BF16, tag="tp", name="tp")
            for mm_ in range(8):
                m = half * 8 + mm_
                nc.tensor.transpose(
                    tp[:, mm_ * 128:(mm_ + 1) * 128],
                    _ap(xp, m, [[16, 128]]), idn[:])
            nc.scalar.activation(
                _ap(dt_, half * 8, [[1, 8], [16, 8], [128, 16]]),
                _ap(tp, 0, [[128, 8], [16, 8], [1, 16]]),
                mybir.ActivationFunctionType.Copy, bias=0.0, scale=1.0)

        # DP in exp domain.  E tiles [128, (g8, jj17)]
        zt = e_pool.tile([128, 136], FP32, tag="tmp", name="tmp")
        nc.vector.memset(zt[:], 0.0)
        e_prev = e_pool.tile([128, 136], FP32, tag="e", name="e")
        nc.vector.tensor_tensor_scan(e_prev[:], dm[:], i0[:], 0.0,
                                     mybir.AluOpType.mult, mybir.AluOpType.add)
        a_t = e_pool.tile([128, 136], FP32, tag="a", name="a")
        for i in range(T):
            nc.vector.tensor_tensor(
                _ap(zt, 1, [[17, 8], [1, 16]]),
                _ap(dt_, i * 128, [[16, 8], [1, 16]]),
                _ap(e_prev, 0, [[17, 8], [1, 16]]),
                mybir.AluOpType.mult)
            nc.vector.scalar_tensor_tensor(
                a_t[:], e_prev[:], D_COEF, zt[:],
                mybir.AluOpType.mult, mybir.AluOpType.add)
            e_new = e_pool.tile([128, 136], FP32, tag="e", name="e")
            nc.vector.tensor_tensor_scan(e_new[:], dm[:], a_t[:], 0.0,
                                         mybir.AluOpType.mult, mybir.AluOpType.add)
            e_prev = e_new

        # loss = -ln(E[16,16]) + 10 - 10*fc
        lne = fin_pool.tile([128, 8], FP32, tag="lne", name="lne")
        nc.scalar.activation(
            lne[:],
            _ap(e_prev, 16, [[17, 8]]),
            mybir.ActivationFunctionType.Ln, bias=0.0, scale=1.0)
        t1 = fin_pool.tile([128, 8], FP32, tag="t1", name="t1")
        nc.vector.tensor_scalar(t1[:], fct[:], -10.0, 10.0,
                                mybir.AluOpType.mult, mybir.AluOpType.add)
        res = fin_pool.tile([128, 8], FP32, tag="res", name="res")
        nc.vector.tensor_tensor(res[:], t1[:], lne[:], mybir.AluOpType.subtract)
        nc.sync.dma_start(out[:], res[:])

    nc.finalize()
    return nc


def _host_prep(tail_logits, target_idx, phon_cost):
    l = np.asarray(tail_logits, dtype=np.float32)
    tidx = np.asarray(target_idx)
    C = np.asarray(phon_cost, dtype=np.float32)
    f8 = ml_dtypes.float8_e4m3

    lmax = l.max(axis=-1, keepdims=True)
    e = np.exp(l - lmax)
    p = e / e.sum(axis=-1, keepdims=True)   # softmax probabilities [B,T,V]

    # fp8 pack of gathered phon-cost columns: cc[v, b*16+j] = 128*C[:,tidx[b,j]]
    C8 = (C * SC).astype(f8)                # [V,V] fp8, exact gather below
    cc = C8[:, tidx].reshape(V, B * J)      # [V, B*16]

    # first-char probability, exact fp32
    fc = p[np.arange(B), 0, tidx[:, 0]].astype(np.float32)

    # masks
    bmask = np.zeros((128, RPB * REG), dtype=np.float32)
    for pp in range(128):
        q = (pp // 16) % 4
        for s in range(RPB):
            bmask[pp, s * REG + q * J:s * REG + (q + 1) * J] = 1.0
    dmask = np.zeros((128, 136), dtype=np.float32)
    init0 = np.zeros((128, 136), dtype=np.float32)
    for g in range(8):
        dmask[:, g * 17 + 1:(g + 1) * 17] = D_COEF
        init0[:, g * 17] = 1.0
    ident = np.eye(128, dtype=np.float32).astype(ml_dtypes.bfloat16)

    in_maps = []
    for k in range(N_CORES):
        sl = slice(k * BSH, (k + 1) * BSH)
        lt = np.ascontiguousarray(
            (p[sl] * SC).transpose(2, 0, 1).reshape(V, BT)).astype(f8)
        ccsh = cc[:, k * BSH * J:(k + 1) * BSH * J]
        in_maps.append({
            "lt0": np.ascontiguousarray(lt[:128]),
            "lt1": np.ascontiguousarray(lt[128:]),
            "cc0": np.ascontiguousarray(ccsh[:128]),
            "cc1": np.ascontiguousarray(ccsh[128:]),
            "bmask": bmask, "dmask": dmask, "init0": init0, "ident": ident,
            "fcin": np.ascontiguousarray(fc[sl].reshape(128, 8)),
        })
    return in_maps


def kernel(tail_logits, target_idx, phon_cost):
    if "nc" not in _cache:
        _cache["nc"] = _build_nc()
    nc = _cache["nc"]
    in_maps = _host_prep(tail_logits, target_idx, phon_cost)
    res = run_bass_kernel_spmd(nc, in_maps, core_ids=list(range(N_CORES)))
    outs = [res.results[k]["out"].reshape(BSH) for k in range(N_CORES)]
    return np.concatenate(outs).astype(np.float32)


# revision 7
# speedup vs baseline: 1.2945x; 1.0847x over previous
"""Trainium2 Bass kernel for the rhyme soft-DP loss (CharLSTMLanguageModelPack).

loss[b] = softDP(sub[b]) + 10*(1 - p[b,0,tidx[b,0]])
  p = softmax(tail_logits, -1); sub[b,t,m] = sum_v p[b,t,v] * C[v, tidx[b,m]]
  softDP: dp[i,j] = softmin(dp[i-1,j]+10, dp[i,j-1]+10, dp[i-1,j-1]+sub[i-1,j-1])
  with softmin(a,b,c) = -log(e^-a + e^-b + e^-c)  (gamma=1)

Device strategy (pure data parallel over B, 1024 pairs/core):
  - Host sends softmax probabilities (x128) and gathered phon-cost packs
    (x128) as fp8-e4m3, transposed: lt[v,(b,t)], cc[v,(b,j)] j<16.
    First-char prob is sent exact (fp32, in the const pack).
  - PE: per 8-pair tile, two 64-row "pack-4 all-pairs" matmuls per
    v-half: ps[64=(4b,16t), 64=(4b',16j)] accumulated over v.
  - ACT drains PSUM with exp: X = exp(-ps/16384) in (0,1].  Since
    exp(-sum) = prod(exp), the diagonal-block extraction becomes
    max(X, 1-mask) (unwanted -> 1) followed by a product tree over b'
    on DVE -- all contiguous bf16 2x-mode ops.  S = exp(-sub) lands
    directly in xp; no separate gall/exp stage.
  - 16 PE transposes + 2 parallel PSUM->SBUF copies (ACT + DVE) give
    D[c, (i,g,j)] f32.
  - exp-domain DP: E[i,j] = d*(E[i-1,j]+E[i,j-1]) + S*E[i-1,j-1],
    d = e^-10, via tensor_tensor_scan rows; two independent interleaved
    chains (pairs g0-3 / g4-7) hide the DVE inter-op bubbles.
  - loss = -ln(E[16,16]) + 10 - 10*p_firstchar.
"""
import numpy as np
import ml_dtypes
from contextlib import ExitStack

import concourse.bass as bass
import concourse.tile as tile
from concourse import bacc, mybir
from concourse.bass_utils import run_bass_kernel_spmd

AP = bass.AP
FP32 = mybir.dt.float32
BF16 = mybir.dt.bfloat16
FP8 = mybir.dt.float8e4

N_CORES = 8
B, T, M, V = 8192, 16, 16, 256
BSH = B // N_CORES            # 1024 pairs per core
NT = BSH // 8                 # 128 tiles of 8 pairs
BT = BSH * T                  # 16384 bt columns per core
J = 16                        # sub cols per pair (first-char handled on host)
REG = 4 * J                   # 64 cols per pack-4 region
RPB = 16                      # regions (tiles) per extract batch: ps [128,1024]
NB = NT // RPB                # 8 batches
CHUNKS = [1024, 1024, 2048, 4096, 4096, 4096]   # input DMA chunk cols
INS_DEL = 10.0
D_COEF = float(np.exp(-INS_DEL))
SC = 128.0                    # fp8 scale for both p and C
ISC2 = 1.0 / (SC * SC)

_cache = {}

def _ap(t, off, dims):
    """Strided free-dim view of a tile: canonical partition dim + custom free dims."""
    base = t[:]
    return AP(base.tensor, base.offset + off, [list(base.ap[0])] + [list(d) for d in dims])


def _apv(base, off, dims):
    """Same, but starting from an AP instead of a tile."""
    return AP(base.tensor, base.offset + off, [list(base.ap[0])] + [list(d) for d in dims])


def _build_nc():
    nc = bacc.Bacc("TRN2", target_bir_lowering=False, debug=False,
                   num_devices=N_CORES)
    lt0 = nc.dram_tensor("lt0", [128, BT], FP8, kind="ExternalInput")
    lt1 = nc.dram_tensor("lt1", [128, BT], FP8, kind="ExternalInput")
    cc0 = nc.dram_tensor("cc0", [128, BSH * J], FP8, kind="ExternalInput")
    cc1 = nc.dram_tensor("cc1", [128, BSH * J], FP8, kind="ExternalInput")
    # fp32 const pack: dm[136] | e0[136] | fc[8]
    cpak = nc.dram_tensor("cpak", [128, 280], FP32, kind="ExternalInput")
    # bf16 const pack: bmneg[1024] | ident[128]
    bpak = nc.dram_tensor("bpak", [128, 1152], BF16, kind="ExternalInput")
    out = nc.dram_tensor("out", [128, 8], FP32, kind="ExternalOutput")

    with tile.TileContext(nc) as tc, ExitStack() as ctx:
        P = lambda name, bufs, **kw: ctx.enter_context(
            tc.tile_pool(name=name, bufs=bufs, **kw))
        const_pool = P("const", 1)
        pt_pool = P("pt", 1)
        ps_pool = P("ps", 3, space="PSUM")
        x_pool = P("x", 3)
        xp_pool = P("xp", 1)
        tp_pool = P("tp", 2, space="PSUM")
        d_pool = P("d", 1)
        e_pool = P("e", 6)
        fin_pool = P("fin", 1)

        # ---- DMA: consts first on sync, inputs split sync/gpsimd rings ----
        cpk = const_pool.tile([128, 280], FP32, tag="cpk", name="cpk")
        nc.sync.dma_start(cpk[:], cpak[:])
        bpk = const_pool.tile([128, 1152], BF16, tag="bpk", name="bpk")
        nc.sync.dma_start(bpk[:], bpak[:])

        pt = [pt_pool.tile([128, BT], FP8, tag=f"pt{h}", name=f"pt{h}")
              for h in range(2)]
        ccs = [pt_pool.tile([128, BSH * J], FP8, tag=f"cc{h}", name=f"cc{h}")
               for h in range(2)]
        off = 0
        for ch in CHUNKS:
            sl = slice(off, off + ch)
            nc.sync.dma_start(pt[0][:, sl], lt0[:, sl])
            nc.gpsimd.dma_start(pt[1][:, sl], lt1[:, sl])
            nc.sync.dma_start(ccs[0][:, sl], cc0[:, sl])
            nc.gpsimd.dma_start(ccs[1][:, sl], cc1[:, sl])
            off += ch

        # const views
        dmA = _ap(cpk, 0, [[1, 68]])
        dmB = _ap(cpk, 68, [[1, 68]])
        e0A = _ap(cpk, 136, [[1, 68]])
        e0B = _ap(cpk, 204, [[1, 68]])
        fct = _ap(cpk, 272, [[1, 8]])
        bmn = _ap(bpk, 0, [[1, 1024]])
        idn = _ap(bpk, 1024, [[1, 128]])

        # t1 = 10 - 10*fc, early (only needs cpk)
        t1 = fin_pool.tile([128, 8], FP32, tag="t1", name="t1")
        nc.vector.tensor_scalar(t1[:], fct, -10.0, 10.0,
                                mybir.AluOpType.mult, mybir.AluOpType.add)

        # S = exp(-sub), bf16, [128=(g,t), (c,j)]
        xp = xp_pool.tile([128, NT * J], BF16, tag="xp", name="xp")

        # ---- matmul + exp-drain + masked product tree, per 16-tile batch ----
        for bi in range(NB):
            c0 = bi * RPB
            ps = ps_pool.tile([128, 1024], FP32, tag="ps", name="ps")
            for s in range(RPB):
                c = c0 + s
                for hh in range(2):       # partition half = 4-pair quad pack
                    for vh in range(2):   # contraction halves over v
                        nc.tensor.matmul(
                            ps[64 * hh:64 * hh + 64, REG * s:REG * s + REG],
                            pt[vh][:, c * 128 + 64 * hh: c * 128 + 64 * hh + 64],
                            ccs[vh][:, (c * 8 + 4 * hh) * J:
                                    (c * 8 + 4 * hh) * J + REG],
                            start=(vh == 0), stop=(vh == 1))
            # ACT: X = exp(-ps/16384)  (PSUM -> SBUF bf16)
            xa = x_pool.tile([128, 1024], BF16, tag="xa", name="xa")
            nc.scalar.activation(xa[:], ps[:],
                                 mybir.ActivationFunctionType.Exp,
                                 bias=0.0, scale=-ISC2)
            # DVE: mask (unwanted -> 1) then product over the 4 b' packs
            xm = x_pool.tile([128, 1024], BF16, tag="xm", name="xm")
            nc.vector.tensor_tensor(xm[:], xa[:], bmn, mybir.AluOpType.max)
            m1 = x_pool.tile([128, 512], BF16, tag="m1", name="m1")
            nc.vector.tensor_tensor(
                m1[:],
                _ap(xm, 0, [[REG, RPB], [1, 32]]),
                _ap(xm, 32, [[REG, RPB], [1, 32]]),
                mybir.AluOpType.mult)
            nc.vector.tensor_tensor(
                _ap(xp, c0 * J, [[J, RPB], [1, J]]),
                _ap(m1, 0, [[32, RPB], [1, J]]),
                _ap(m1, 16, [[32, RPB], [1, J]]),
                mybir.AluOpType.mult)

        # ---- S rearrange: 16 PE transposes -> D[c, (i,g,j)] f32 ----
        dt_ = d_pool.tile([128, 16 * 128], FP32, tag="dt", name="dt")
        tps = []
        for half in range(2):
            tp = tp_pool.tile([128, 1024], BF16, tag="tp", name="tp")
            tps.append(tp)
            for mm_ in range(8):
                m = half * 8 + mm_
                nc.tensor.transpose(
                    tp[:, mm_ * 128:(mm_ + 1) * 128],
                    _ap(xp, m, [[16, 128]]), idn)
        # two PSUM->SBUF copies run concurrently on ACT and DVE
        nc.scalar.activation(
            _ap(dt_, 0, [[1, 8], [16, 8], [128, 16]]),
            _ap(tps[0], 0, [[128, 8], [16, 8], [1, 16]]),
            mybir.ActivationFunctionType.Copy, bias=0.0, scale=1.0)
        nc.vector.tensor_copy(
            _ap(dt_, 8, [[1, 8], [16, 8], [128, 16]]),
            _ap(tps[1], 0, [[128, 8], [16, 8], [1, 16]]))

        # ---- DP in exp domain: two interleaved chains (g0-3 | g4-7) ----
        # E tiles [128, (g4, jj17)]; chain X handles pairs 4X..4X+3.
        zt = [e_pool.tile([128, 68], FP32, tag=f"z{x}", name=f"z{x}")
              for x in range(2)]
        for x in range(2):
            nc.vector.memset(zt[x][:], 0.0)
        e_prev = [e0A, e0B]
        dms = [dmA, dmB]
        a_t = [e_pool.tile([128, 68], FP32, tag=f"a{x}", name=f"a{x}")
               for x in range(2)]
        for i in range(T):
            for x in range(2):
                nc.vector.tensor_tensor(
                    _ap(zt[x], 1, [[17, 4], [1, 16]]),
                    _ap(dt_, i * 128 + x * 64, [[16, 4], [1, 16]]),
                    _apv(e_prev[x], 0, [[17, 4], [1, 16]]),
                    mybir.AluOpType.mult)
            for x in range(2):
                nc.vector.scalar_tensor_tensor(
                    a_t[x][:], e_prev[x], D_COEF, zt[x][:],
                    mybir.AluOpType.mult, mybir.AluOpType.add)
            new = []
            for x in range(2):
                e_new = e_pool.tile([128, 68], FP32, tag=f"e{x}", name=f"e{x}")
                nc.vector.tensor_tensor_scan(e_new[:], dms[x], a_t[x][:], 0.0,
                                             mybir.AluOpType.mult,
                                             mybir.AluOpType.add)
                new.append(e_new[:])
            e_prev = new

        # ---- loss = -ln(E[16,16]) + 10 - 10*fc ----
        lne = fin_pool.tile([128, 8], FP32, tag="lne", name="lne")
        for x in range(2):
            nc.scalar.activation(
                lne[:, 4 * x:4 * x + 4],
                _apv(e_prev[x], 16, [[17, 4]]),
                mybir.ActivationFunctionType.Ln, bias=0.0, scale=1.0)
        res = fin_pool.tile([128, 8], FP32, tag="res", name="res")
        nc.vector.tensor_tensor(res[:], t1[:], lne[:], mybir.AluOpType.subtract)
        nc.sync.dma_start(out[:], res[:])

    nc.finalize()
    return nc


def _host_prep(tail_logits, target_idx, phon_cost):
    l = np.asarray(tail_logits, dtype=np.float32)
    tidx = np.asarray(target_idx)
    C = np.asarray(phon_cost, dtype=np.float32)
    f8 = ml_dtypes.float8_e4m3

    lmax = l.max(axis=-1, keepdims=True)
    e = np.exp(l - lmax)
    p = e / e.sum(axis=-1, keepdims=True)   # softmax probabilities [B,T,V]

    # fp8 pack of gathered phon-cost columns: cc[v, b*16+j] = 128*C[:,tidx[b,j]]
    C8 = (C * SC).astype(f8)                # [V,V] fp8, exact gather below
    cc = C8[:, tidx].reshape(V, B * J)      # [V, B*16]

    # first-char probability, exact fp32
    fc = p[np.arange(B), 0, tidx[:, 0]].astype(np.float32)

    # fp32 const pack: dm[136] | e0[136] | fc[8] (fc per core below)
    dmask = np.zeros(136, dtype=np.float32)
    e0 = np.zeros(136, dtype=np.float32)
    for g in range(8):
        dmask[g * 17 + 1:(g + 1) * 17] = D_COEF
        e0[g * 17:(g + 1) * 17] = D_COEF ** np.arange(17, dtype=np.float32)
    cbase = np.zeros((128, 280), dtype=np.float32)
    cbase[:, 0:136] = dmask
    cbase[:, 136:272] = e0

    # bf16 const pack: bmneg[1024] | ident[128]
    bpak = np.zeros((128, 1152), dtype=np.float32)
    for pp in range(128):
        q = (pp // 16) % 4
        for s in range(RPB):
            for b4 in range(4):
                if b4 != q:
                    bpak[pp, s * REG + b4 * J:s * REG + (b4 + 1) * J] = 1.0
    bpak[:, 1024:1152] = np.eye(128, dtype=np.float32)
    bpak = bpak.astype(ml_dtypes.bfloat16)

    in_maps = []
    for k in range(N_CORES):
        sl = slice(k * BSH, (k + 1) * BSH)
        lt = np.ascontiguousarray(
            (p[sl] * SC).transpose(2, 0, 1).reshape(V, BT)).astype(f8)
        ccsh = cc[:, k * BSH * J:(k + 1) * BSH * J]
        cpak = cbase.copy()
        cpak[:, 272:280] = fc[sl].reshape(128, 8)
        in_maps.append({
            "lt0": np.ascontiguousarray(lt[:128]),
            "lt1": np.ascontiguousarray(lt[128:]),
            "cc0": np.ascontiguousarray(ccsh[:128]),
            "cc1": np.ascontiguousarray(ccsh[128:]),
            "cpak": cpak, "bpak": bpak,
        })
    return in_maps


def kernel(tail_logits, target_idx, phon_cost):
    if "nc" not in _cache:
        _cache["nc"] = _build_nc()
    nc = _cache["nc"]
    in_maps = _host_prep(tail_logits, target_idx, phon_cost)
    res = run_bass_kernel_spmd(nc, in_maps, core_ids=list(range(N_CORES)))
    outs = [res.results[k]["out"].reshape(BSH) for k in range(N_CORES)]
    return np.concatenate(outs).astype(np.float32)


# revision 8
# speedup vs baseline: 1.3328x; 1.0296x over previous
"""Trainium2 Bass kernel for the rhyme soft-DP loss (CharLSTMLanguageModelPack).

loss[b] = softDP(sub[b]) + 10*(1 - p[b,0,tidx[b,0]])
  p = softmax(tail_logits, -1); sub[b,t,m] = sum_v p[b,t,v] * C[v, tidx[b,m]]
  softDP: dp[i,j] = softmin(dp[i-1,j]+10, dp[i,j-1]+10, dp[i-1,j-1]+sub[i-1,j-1])
  with softmin(a,b,c) = -log(e^-a + e^-b + e^-c)  (gamma=1)

Device strategy (pure data parallel over B, 1024 pairs/core):
  - Host sends softmax probabilities (x128) and gathered phon-cost packs
    (x128) as fp8-e4m3, transposed: lt[v,(b,t)], cc[v,(b,j)] j<16.
    First-char prob is sent exact (fp32, in the const pack).
  - PE: per 8-pair tile, two 64-row "pack-4 all-pairs" matmuls per
    v-half: ps[64=(4b,16t), 64=(4b',16j)] accumulated over v.
  - ACT drains PSUM with exp: X = exp(-ps/16384) in (0,1].  Since
    exp(-sum) = prod(exp), the diagonal-block extraction becomes
    max(X, 1-mask) (unwanted -> 1) followed by a product tree over b'
    on DVE -- all contiguous bf16 2x-mode ops.  S = exp(-sub) lands
    directly in xp; no separate gall/exp stage.
  - 16 PE transposes + 2 parallel PSUM->SBUF copies (ACT + DVE) give
    D[c, (i,g,j)] f32.
  - exp-domain DP: E[i,j] = d*(E[i-1,j]+E[i,j-1]) + S*E[i-1,j-1],
    d = e^-10, via tensor_tensor_scan rows; two independent interleaved
    chains (pairs g0-3 / g4-7) hide the DVE inter-op bubbles.
  - loss = -ln(E[16,16]) + 10 - 10*p_firstchar.
"""
import numpy as np
import ml_dtypes
from contextlib import ExitStack

import concourse.bass as bass
import concourse.tile as tile
from concourse import bacc, mybir
from concourse.bass_utils import run_bass_kernel_spmd

AP = bass.AP
FP32 = mybir.dt.float32
BF16 = mybir.dt.bfloat16
FP8 = mybir.dt.float8e4

N_CORES = 8
B, T, M, V = 8192, 16, 16, 256
BSH = B // N_CORES            # 1024 pairs per core
NT = BSH // 8                 # 128 tiles of 8 pairs
BT = BSH * T                  # 16384 bt columns per core
J = 16                        # sub cols per pair (first-char handled on host)
REG = 4 * J                   # 64 cols per pack-4 region
RPB = 16                      # regions (tiles) per extract batch: ps [128,1024]
NB = NT // RPB                # 8 batches
CHUNKS = [1024, 1024, 2048, 4096, 4096, 4096]   # input DMA chunk cols
INS_DEL = 10.0
D_COEF = float(np.exp(-INS_DEL))
SC = 128.0                    # fp8 scale for both p and C
ISC2 = 1.0 / (SC * SC)

_cache = {}

def _ap(t, off, dims):
    """Strided free-dim view of a tile: canonical partition dim + custom free dims."""
    base = t[:]
    return AP(base.tensor, base.offset + off, [list(base.ap[0])] + [list(d) for d in dims])


def _apv(base, off, dims):
    """Same, but starting from an AP instead of a tile."""
    return AP(base.tensor, base.offset + off, [list(base.ap[0])] + [list(d) for d in dims])


def _build_nc():
    nc = bacc.Bacc("TRN2", target_bir_lowering=False, debug=False,
                   num_devices=N_CORES)
    lt0 = nc.dram_tensor("lt0", [128, BT], FP8, kind="ExternalInput")
    lt1 = nc.dram_tensor("lt1", [128, BT], FP8, kind="ExternalInput")
    cc0 = nc.dram_tensor("cc0", [128, BSH * J], FP8, kind="ExternalInput")
    cc1 = nc.dram_tensor("cc1", [128, BSH * J], FP8, kind="ExternalInput")
    # fp32 const pack: dm[136] | e0[136] | fc[8]
    cpak = nc.dram_tensor("cpak", [128, 280], FP32, kind="ExternalInput")
    # bf16 const pack: bmneg[1024] | ident[128]
    bpak = nc.dram_tensor("bpak", [128, 1152], BF16, kind="ExternalInput")
    out = nc.dram_tensor("out", [128, 8], FP32, kind="ExternalOutput")

    with tile.TileContext(nc) as tc, ExitStack() as ctx:
        P = lambda name, bufs, **kw: ctx.enter_context(
            tc.tile_pool(name=name, bufs=bufs, **kw))
        const_pool = P("const", 1)
        pt_pool = P("pt", 1)
        ps_pool = P("ps", 3, space="PSUM")
        x_pool = P("x", 3)
        xp_pool = P("xp", 1)
        tp_pool = P("tp", 2, space="PSUM")
        d_pool = P("d", 1)
        e_pool = P("e", 6)
        fin_pool = P("fin", 1)

        # ---- DMA: consts first on sync, inputs split sync/gpsimd rings ----
        cpk = const_pool.tile([128, 280], FP32, tag="cpk", name="cpk")
        nc.sync.dma_start(cpk[:], cpak[:])
        bpk = const_pool.tile([128, 1152], BF16, tag="bpk", name="bpk")
        nc.sync.dma_start(bpk[:], bpak[:])

        pt = [pt_pool.tile([128, BT], FP8, tag=f"pt{h}", name=f"pt{h}")
              for h in range(2)]
        ccs = [pt_pool.tile([128, BSH * J], FP8, tag=f"cc{h}", name=f"cc{h}")
               for h in range(2)]
        off = 0
        for ch in CHUNKS:
            sl = slice(off, off + ch)
            nc.sync.dma_start(pt[0][:, sl], lt0[:, sl])
            nc.sync.dma_start(pt[1][:, sl], lt1[:, sl])
            nc.sync.dma_start(ccs[0][:, sl], cc0[:, sl])
            nc.sync.dma_start(ccs[1][:, sl], cc1[:, sl])
            off += ch

        # const views
        dmA = _ap(cpk, 0, [[1, 68]])
        dmB = _ap(cpk, 68, [[1, 68]])
        e0A = _ap(cpk, 136, [[1, 68]])
        e0B = _ap(cpk, 204, [[1, 68]])
        fct = _ap(cpk, 272, [[1, 8]])
        bmn = _ap(bpk, 0, [[1, 1024]])
        idn = _ap(bpk, 1024, [[1, 128]])

        # t1 = 10 - 10*fc, early (only needs cpk)
        t1 = fin_pool.tile([128, 8], FP32, tag="t1", name="t1")
        nc.vector.tensor_scalar(t1[:], fct, -10.0, 10.0,
                                mybir.AluOpType.mult, mybir.AluOpType.add)

        # S = exp(-sub), bf16, [128=(g,t), (c,j)]
        xp = xp_pool.tile([128, NT * J], BF16, tag="xp", name="xp")

        # ---- matmul + exp-drain + masked product tree, per 16-tile batch ----
        for bi in range(NB):
            c0 = bi * RPB
            ps = ps_pool.tile([128, 1024], FP32, tag="ps", name="ps")
            for s in range(RPB):
                c = c0 + s
                for hh in range(2):       # partition half = 4-pair quad pack
                    for vh in range(2):   # contraction halves over v
                        nc.tensor.matmul(
                            ps[64 * hh:64 * hh + 64, REG * s:REG * s + REG],
                            pt[vh][:, c * 128 + 64 * hh: c * 128 + 64 * hh + 64],
                            ccs[vh][:, (c * 8 + 4 * hh) * J:
                                    (c * 8 + 4 * hh) * J + REG],
                            start=(vh == 0), stop=(vh == 1))
            # ACT: X = exp(-ps/16384)  (PSUM -> SBUF bf16)
            xa = x_pool.tile([128, 1024], BF16, tag="xa", name="xa")
            nc.scalar.activation(xa[:], ps[:],
                                 mybir.ActivationFunctionType.Exp,
                                 bias=0.0, scale=-ISC2)
            # DVE: mask (unwanted -> 1) then product over the 4 b' packs
            xm = x_pool.tile([128, 1024], BF16, tag="xm", name="xm")
            nc.vector.tensor_tensor(xm[:], xa[:], bmn, mybir.AluOpType.max)
            m1 = x_pool.tile([128, 512], BF16, tag="m1", name="m1")
            nc.vector.tensor_tensor(
                m1[:],
                _ap(xm, 0, [[REG, RPB], [1, 32]]),
                _ap(xm, 32, [[REG, RPB], [1, 32]]),
                mybir.AluOpType.mult)
            nc.vector.tensor_tensor(
                _ap(xp, c0 * J, [[J, RPB], [1, J]]),
                _ap(m1, 0, [[32, RPB], [1, J]]),
                _ap(m1, 16, [[32, RPB], [1, J]]),
                mybir.AluOpType.mult)

        # ---- S rearrange: 16 PE transposes -> D[c, (i,g,j)] f32 ----
        dt_ = d_pool.tile([128, 16 * 128], FP32, tag="dt", name="dt")
        tps = []
        for half in range(2):
            tp = tp_pool.tile([128, 1024], BF16, tag="tp", name="tp")
            tps.append(tp)
            for mm_ in range(8):
                m = half * 8 + mm_
                nc.tensor.transpose(
                    tp[:, mm_ * 128:(mm_ + 1) * 128],
                    _ap(xp, m, [[16, 128]]), idn)
        # two PSUM->SBUF copies run concurrently on ACT and DVE
        nc.scalar.activation(
            _ap(dt_, 0, [[1, 8], [16, 8], [128, 16]]),
            _ap(tps[0], 0, [[128, 8], [16, 8], [1, 16]]),
            mybir.ActivationFunctionType.Copy, bias=0.0, scale=1.0)
        nc.vector.tensor_copy(
            _ap(dt_, 8, [[1, 8], [16, 8], [128, 16]]),
            _ap(tps[1], 0, [[128, 8], [16, 8], [1, 16]]))

        # ---- DP in exp domain: two interleaved chains (g0-3 | g4-7) ----
        # E tiles [128, (g4, jj17)]; chain X handles pairs 4X..4X+3.
        zt = [e_pool.tile([128, 68], FP32, tag=f"z{x}", name=f"z{x}")
              for x in range(2)]
        for x in range(2):
            nc.vector.memset(zt[x][:], 0.0)
        e_prev = [e0A, e0B]
        dms = [dmA, dmB]
        a_t = [e_pool.tile([128, 68], FP32, tag=f"a{x}", name=f"a{x}")
               for x in range(2)]
        for i in range(T):
            for x in range(2):
                nc.vector.tensor_tensor(
                    _ap(zt[x], 1, [[17, 4], [1, 16]]),
                    _ap(dt_, i * 128 + x * 64, [[16, 4], [1, 16]]),
                    _apv(e_prev[x], 0, [[17, 4], [1, 16]]),
                    mybir.AluOpType.mult)
            for x in range(2):
                nc.vector.scalar_tensor_tensor(
                    a_t[x][:], e_prev[x], D_COEF, zt[x][:],
                    mybir.AluOpType.mult, mybir.AluOpType.add)
            new = []
            for x in range(2):
                e_new = e_pool.tile([128, 68], FP32, tag=f"e{x}", name=f"e{x}")
                nc.vector.tensor_tensor_scan(e_new[:], dms[x], a_t[x][:], 0.0,
                                             mybir.AluOpType.mult,
                                             mybir.AluOpType.add)
                new.append(e_new[:])
            e_prev = new

        # ---- loss = -ln(E[16,16]) + 10 - 10*fc ----
        lne = fin_pool.tile([128, 8], FP32, tag="lne", name="lne")
        for x in range(2):
            nc.scalar.activation(
                lne[:, 4 * x:4 * x + 4],
                _apv(e_prev[x], 16, [[17, 4]]),
                mybir.ActivationFunctionType.Ln, bias=0.0, scale=1.0)
        res = fin_pool.tile([128, 8], FP32, tag="res", name="res")
        nc.vector.tensor_tensor(res[:], t1[:], lne[:], mybir.AluOpType.subtract)
        nc.sync.dma_start(out[:], res[:])

    nc.finalize()
    return nc


def _host_prep(tail_logits, target_idx, phon_cost):
    l = np.asarray(tail_logits, dtype=np.float32)
    tidx = np.asarray(target_idx)
    C = np.asarray(phon_cost, dtype=np.float32)
    f8 = ml_dtypes.float8_e4m3

    lmax = l.max(axis=-1, keepdims=True)
    e = np.exp(l - lmax)
    p = e / e.sum(axis=-1, keepdims=True)   # softmax probabilities [B,T,V]

    # fp8 pack of gathered phon-cost columns: cc[v, b*16+j] = 128*C[:,tidx[b,j]]
    C8 = (C * SC).astype(f8)                # [V,V] fp8, exact gather below
    cc = C8[:, tidx].reshape(V, B * J)      # [V, B*16]

    # first-char probability, exact fp32
    fc = p[np.arange(B), 0, tidx[:, 0]].astype(np.float32)

    # fp32 const pack: dm[136] | e0[136] | fc[8] (fc per core below)
    dmask = np.zeros(136, dtype=np.float32)
    e0 = np.zeros(136, dtype=np.float32)
    for g in range(8):
        dmask[g * 17 + 1:(g + 1) * 17] = D_COEF
        e0[g * 17:(g + 1) * 17] = D_COEF ** np.arange(17, dtype=np.float32)
    cbase = np.zeros((128, 280), dtype=np.float32)
    cbase[:, 0:136] = dmask
    cbase[:, 136:272] = e0

    # bf16 const pack: bmneg[1024] | ident[128]
    bpak = np.zeros((128, 1152), dtype=np.float32)
    for pp in range(128):
        q = (pp // 16) % 4
        for s in range(RPB):
            for b4 in range(4):
                if b4 != q:
                    bpak[pp, s * REG + b4 * J:s * REG + (b4 + 1) * J] = 1.0
    bpak[:, 1024:1152] = np.eye(128, dtype=np.float32)
    bpak = bpak.astype(ml_dtypes.bfloat16)

    in_maps = []
    for k in range(N_CORES):
        sl = slice(k * BSH, (k + 1) * BSH)
        lt = np.ascontiguousarray(
            (p[sl] * SC).transpose(2, 0, 1).reshape(V, BT)).astype(f8)
        ccsh = cc[:, k * BSH * J:(k + 1) * BSH * J]
        cpak = cbase.copy()
        cpak[:, 272:280] = fc[sl].reshape(128, 8)
        in_maps.append({
            "lt0": np.ascontiguousarray(lt[:128]),
            "lt1": np.ascontiguousarray(lt[128:]),
            "cc0": np.ascontiguousarray(ccsh[:128]),
            "cc1": np.ascontiguousarray(ccsh[128:]),
            "cpak": cpak, "bpak": bpak,
        })
    return in_maps


def kernel(tail_logits, target_idx, phon_cost):
    if "nc" not in _cache:
        _cache["nc"] = _build_nc()
    nc = _cache["nc"]
    in_maps = _host_prep(tail_logits, target_idx, phon_cost)
    res = run_bass_kernel_spmd(nc, in_maps, core_ids=list(range(N_CORES)))
    outs = [res.results[k]["out"].reshape(BSH) for k in range(N_CORES)]
    return np.concatenate(outs).astype(np.float32)


# revision 14
# speedup vs baseline: 1.4454x; 1.0845x over previous
"""Trainium2 Bass kernel for the rhyme soft-DP loss (CharLSTMLanguageModelPack).

loss[b] = softDP(sub[b]) + 10*(1 - p[b,0,tidx[b,0]])
  p = softmax(tail_logits, -1); sub[b,t,m] = sum_v p[b,t,v] * C[v, tidx[b,m]]
  softDP: dp[i,j] = softmin(dp[i-1,j]+10, dp[i,j-1]+10, dp[i-1,j-1]+sub[i-1,j-1])
  with softmin(a,b,c) = -log(e^-a + e^-b + e^-c)  (gamma=1)

Device strategy (pure data parallel over B, 1024 pairs/core):
  - Host sends softmax probabilities (x128) and gathered phon-cost packs
    (x128) as fp8-e4m3, transposed: lt[v,(b,t)], cc[v,(b,j)] j<16.
    First-char prob is sent exact (fp32, in the const pack).
  - PE: per 8-pair tile, two 64-row "pack-4 all-pairs" matmuls per
    v-half: ps[64=(4b,16t), 64=(4b',16j)] accumulated over v.
  - ACT drains PSUM with exp: X = exp(-ps/16384) in (0,1].  Since
    exp(-sum) = prod(exp), the diagonal-block extraction becomes
    max(X, 1-mask) (unwanted -> 1) followed by a product tree over b'
    on DVE -- all contiguous bf16 2x-mode ops.  S = exp(-sub) lands
    directly in xp; no separate gall/exp stage.
  - 16 PE transposes + 2 parallel PSUM->SBUF copies (ACT + DVE) give
    D[c, (i,g,j)] f32.
  - exp-domain DP: E[i,j] = d*(E[i-1,j]+E[i,j-1]) + S*E[i-1,j-1],
    d = e^-10, via tensor_tensor_scan rows; two independent interleaved
    chains (pairs g0-3 / g4-7) hide the DVE inter-op bubbles.
  - loss = -ln(E[16,16]) + 10 - 10*p_firstchar.
"""
import numpy as np
import ml_dtypes
from contextlib import ExitStack

import concourse.bass as bass
import concourse.tile as tile
from concourse import bacc, mybir
from concourse.bass_utils import run_bass_kernel_spmd

AP = bass.AP
FP32 = mybir.dt.float32
BF16 = mybir.dt.bfloat16
FP8 = mybir.dt.float8e4

N_CORES = 8
B, T, M, V = 8192, 16, 16, 256
BSH = B // N_CORES            # 1024 pairs per core
NT = BSH // 8                 # 128 tiles of 8 pairs
BT = BSH * T                  # 16384 bt columns per core
J = 16                        # sub cols per pair (first-char handled on host)
REG = 4 * J                   # 64 cols per pack-4 region
RPB = 16                      # regions (tiles) per extract batch: ps [128,1024]
NB = NT // RPB                # 8 batches
GRP = 4 * 2048                # blob cols per batch group: lt0|lt1|cc0|cc1 x 2048
INS_DEL = 10.0
D_COEF = float(np.exp(-INS_DEL))
SC = 128.0                    # fp8 scale for both p and C
ISC2 = 1.0 / (SC * SC)

_cache = {}

def _ap(t, off, dims):
    """Strided free-dim view of a tile: canonical partition dim + custom free dims."""
    base = t[:]
    return AP(base.tensor, base.offset + off, [list(base.ap[0])] + [list(d) for d in dims])


def _apv(base, off, dims):
    """Same, but starting from an AP instead of a tile."""
    return AP(base.tensor, base.offset + off, [list(base.ap[0])] + [list(d) for d in dims])


def _build_nc():
    nc = bacc.Bacc("TRN2", target_bir_lowering=False, debug=False,
                   num_devices=N_CORES)
    # all four fp8 inputs, interleaved per 16-tile batch group:
    # [lt0|lt1|cc0|cc1] x 2048 cols each, 8 groups
    blob = nc.dram_tensor("blob", [128, 4 * BT], FP8, kind="ExternalInput")
    # fp32 const pack: dm[136] | e0[136] | fc[8]
    cpak = nc.dram_tensor("cpak", [128, 280], FP32, kind="ExternalInput")
    # bf16 const pack: bmneg[1024] | ident[128]
    bpak = nc.dram_tensor("bpak", [128, 1152], BF16, kind="ExternalInput")
    out = nc.dram_tensor("out", [128, 8], FP32, kind="ExternalOutput")

    with tile.TileContext(nc) as tc, ExitStack() as ctx:
        P = lambda name, bufs, **kw: ctx.enter_context(
            tc.tile_pool(name=name, bufs=bufs, **kw))
        const_pool = P("const", 1)
        pt_pool = P("pt", 1)
        ps_pool = P("ps", 3, space="PSUM")
        x_pool = P("x", 3)
        xp_pool = P("xp", 1)
        tp_pool = P("tp", 2, space="PSUM")
        d_pool = P("d", 1)
        e_pool = P("e", 6)
        fin_pool = P("fin", 1)

        # ---- DMA: consts first on sync, inputs split sync/gpsimd rings ----
        cpk = const_pool.tile([128, 280], FP32, tag="cpk", name="cpk")
        nc.sync.dma_start(cpk[:], cpak[:])
        bpk = const_pool.tile([128, 1152], BF16, tag="bpk", name="bpk")
        nc.sync.dma_start(bpk[:], bpak[:])

        blb = pt_pool.tile([128, 4 * BT], FP8, tag="blb", name="blb")
        for bi in range(NB):
            sl = slice(bi * GRP, (bi + 1) * GRP)
            nc.sync.dma_start(blb[:, sl], blob[:, sl])

        def pt_at(vh, col):      # lt[vh] column -> blob column
            bi, c = divmod(col, 2048)
            return blb[:].offset + bi * GRP + vh * 2048 + c

        def cc_at(vh, col):      # cc[vh] column -> blob column
            bi, c = divmod(col, 2048)
            return blb[:].offset + bi * GRP + (2 + vh) * 2048 + c

        def bview(off, n):
            return AP(blb[:].tensor, off, [list(blb[:].ap[0]), [1, n]])

        # const views
        dmA = _ap(cpk, 0, [[1, 68]])
        dmB = _ap(cpk, 68, [[1, 68]])
        e0A = _ap(cpk, 136, [[1, 68]])
        e0B = _ap(cpk, 204, [[1, 68]])
        fct = _ap(cpk, 272, [[1, 8]])
        bmn = _ap(bpk, 0, [[1, 1024]])
        idn = _ap(bpk, 1024, [[1, 128]])

        # t1 = 10 - 10*fc, early (only needs cpk)
        t1 = fin_pool.tile([128, 8], FP32, tag="t1", name="t1")
        nc.vector.tensor_scalar(t1[:], fct, -10.0, 10.0,
                                mybir.AluOpType.mult, mybir.AluOpType.add)

        # S = exp(-sub), bf16, [128=(g,t), (c,j)]
        xp = xp_pool.tile([128, NT * J], BF16, tag="xp", name="xp")

        # ---- matmul + exp-drain + masked product tree, per 16-tile batch ----
        for bi in range(NB):
            c0 = bi * RPB
            ps = ps_pool.tile([128, 1024], FP32, tag="ps", name="ps")
            for s in range(RPB):
                c = c0 + s
                for hh in range(2):       # partition half = 4-pair quad pack
                    for vh in range(2):   # contraction halves over v
                        nc.tensor.matmul(
                            ps[64 * hh:64 * hh + 64, REG * s:REG * s + REG],
                            bview(pt_at(vh, c * 128 + 64 * hh), 64),
                            bview(cc_at(vh, (c * 8 + 4 * hh) * J), REG),
                            start=(vh == 0), stop=(vh == 1))
            # ACT: X = exp(-ps/16384)  (PSUM -> SBUF bf16)
            xa = x_pool.tile([128, 1024], BF16, tag="xa", name="xa")
            nc.scalar.activation(xa[:], ps[:],
                                 mybir.ActivationFunctionType.Exp,
                                 bias=0.0, scale=-ISC2)
            # DVE: mask (unwanted -> 1) then product over the 4 b' packs
            xm = x_pool.tile([128, 1024], BF16, tag="xm", name="xm")
            nc.vector.tensor_tensor(xm[:], xa[:], bmn, mybir.AluOpType.max)
            m1 = x_pool.tile([128, 512], BF16, tag="m1", name="m1")
            nc.vector.tensor_tensor(
                m1[:],
                _ap(xm, 0, [[REG, RPB], [1, 32]]),
                _ap(xm, 32, [[REG, RPB], [1, 32]]),
                mybir.AluOpType.mult)
            nc.vector.tensor_tensor(
                _ap(xp, c0 * J, [[J, RPB], [1, J]]),
                _ap(m1, 0, [[32, RPB], [1, J]]),
                _ap(m1, 16, [[32, RPB], [1, J]]),
                mybir.AluOpType.mult)

        # ---- S rearrange: 16 PE transposes -> D[c, (i,g,j)] f32 ----
        dt_ = d_pool.tile([128, 16 * 128], FP32, tag="dt", name="dt")
        tps = []
        for half in range(2):
            tp = tp_pool.tile([128, 1024], BF16, tag="tp", name="tp")
            tps.append(tp)
            for mm_ in range(8):
                m = half * 8 + mm_
                nc.tensor.transpose(
                    tp[:, mm_ * 128:(mm_ + 1) * 128],
                    _ap(xp, m, [[16, 128]]), idn)
        # two PSUM->SBUF copies run concurrently on ACT and DVE
        nc.scalar.activation(
            _ap(dt_, 0, [[1, 8], [16, 8], [128, 16]]),
            _ap(tps[0], 0, [[128, 8], [16, 8], [1, 16]]),
            mybir.ActivationFunctionType.Copy, bias=0.0, scale=1.0)
        nc.vector.tensor_copy(
            _ap(dt_, 8, [[1, 8], [16, 8], [128, 16]]),
            _ap(tps[1], 0, [[128, 8], [16, 8], [1, 16]]))

        # ---- DP in exp domain: two interleaved chains (g0-3 | g4-7) ----
        # E tiles [128, (g4, jj17)]; chain X handles pairs 4X..4X+3.
        zt = [e_pool.tile([128, 68], FP32, tag=f"z{x}", name=f"z{x}")
              for x in range(2)]
        for x in range(2):
            nc.vector.memset(zt[x][:], 0.0)
        e_prev = [e0A, e0B]
        dms = [dmA, dmB]
        a_t = [e_pool.tile([128, 68], FP32, tag=f"a{x}", name=f"a{x}")
               for x in range(2)]
        for i in range(T):
            for x in range(2):
                nc.vector.tensor_tensor(
                    _ap(zt[x], 1, [[17, 4], [1, 16]]),
                    _ap(dt_, i * 128 + x * 64, [[16, 4], [1, 16]]),
                    _apv(e_prev[x], 0, [[17, 4], [1, 16]]),
                    mybir.AluOpType.mult)
            for x in range(2):
                nc.vector.scalar_tensor_tensor(
                    a_t[x][:], e_prev[x], D_COEF, zt[x][:],
                    mybir.AluOpType.mult, mybir.AluOpType.add)
            new = []
            for x in range(2):
                e_new = e_pool.tile([128, 68], FP32, tag=f"e{x}", name=f"e{x}")
                nc.vector.tensor_tensor_scan(e_new[:], dms[x], a_t[x][:], 0.0,
                                             mybir.AluOpType.mult,
                                             mybir.AluOpType.add)
                new.append(e_new[:])
            e_prev = new

        # ---- loss = -ln(E[16,16]) + 10 - 10*fc ----
        lne = fin_pool.tile([128, 8], FP32, tag="lne", name="lne")
        for x in range(2):
            nc.scalar.activation(
                lne[:, 4 * x:4 * x + 4],
                _apv(e_prev[x], 16, [[17, 4]]),
                mybir.ActivationFunctionType.Ln, bias=0.0, scale=1.0)
        res = fin_pool.tile([128, 8], FP32, tag="res", name="res")
        nc.vector.tensor_tensor(res[:], t1[:], lne[:], mybir.AluOpType.subtract)
        nc.sync.dma_start(out[:], res[:])

    nc.finalize()
    return nc


def _host_prep(tail_logits, target_idx, phon_cost):
    l = np.asarray(tail_logits, dtype=np.float32)
    tidx = np.asarray(target_idx)
    C = np.asarray(phon_cost, dtype=np.float32)
    f8 = ml_dtypes.float8_e4m3

    lmax = l.max(axis=-1, keepdims=True)
    e = np.exp(l - lmax)
    p = e / e.sum(axis=-1, keepdims=True)   # softmax probabilities [B,T,V]

    # fp8 pack of gathered phon-cost columns: cc[v, b*16+j] = 128*C[:,tidx[b,j]]
    C8 = (C * SC).astype(f8)                # [V,V] fp8, exact gather below
    cc = C8[:, tidx].reshape(V, B * J)      # [V, B*16]

    # first-char probability, exact fp32
    fc = p[np.arange(B), 0, tidx[:, 0]].astype(np.float32)

    # fp32 const pack: dm[136] | e0[136] | fc[8] (fc per core below)
    dmask = np.zeros(136, dtype=np.float32)
    e0 = np.zeros(136, dtype=np.float32)
    for g in range(8):
        dmask[g * 17 + 1:(g + 1) * 17] = D_COEF
        e0[g * 17:(g + 1) * 17] = D_COEF ** np.arange(17, dtype=np.float32)
    cbase = np.zeros((128, 280), dtype=np.float32)
    cbase[:, 0:136] = dmask
    cbase[:, 136:272] = e0

    # bf16 const pack: bmneg[1024] | ident[128]
    bpak = np.zeros((128, 1152), dtype=np.float32)
    for pp in range(128):
        q = (pp // 16) % 4
        for s in range(RPB):
            for b4 in range(4):
                if b4 != q:
                    bpak[pp, s * REG + b4 * J:s * REG + (b4 + 1) * J] = 1.0
    bpak[:, 1024:1152] = np.eye(128, dtype=np.float32)
    bpak = bpak.astype(ml_dtypes.bfloat16)

    in_maps = []
    for k in range(N_CORES):
        sl = slice(k * BSH, (k + 1) * BSH)
        lt = np.ascontiguousarray(
            (p[sl] * SC).transpose(2, 0, 1).reshape(V, BT)).astype(f8)
        ccsh = cc[:, k * BSH * J:(k + 1) * BSH * J]
        # blob: per 2048-col batch group, [lt0 | lt1 | cc0 | cc1]
        blob = np.empty((128, 4 * BT), dtype=f8)
        g = blob.reshape(128, NB, 4, 2048)
        g[:, :, 0, :] = lt[:128].reshape(128, NB, 2048)
        g[:, :, 1, :] = lt[128:].reshape(128, NB, 2048)
        g[:, :, 2, :] = ccsh[:128].reshape(128, NB, 2048)
        g[:, :, 3, :] = ccsh[128:].reshape(128, NB, 2048)
        cpak = cbase.copy()
        cpak[:, 272:280] = fc[sl].reshape(128, 8)
        in_maps.append({"blob": blob, "cpak": cpak, "bpak": bpak})
    return in_maps


def kernel(tail_logits, target_idx, phon_cost):
    if "nc" not in _cache:
        _cache["nc"] = _build_nc()
    nc = _cache["nc"]
    in_maps = _host_prep(tail_logits, target_idx, phon_cost)
    res = run_bass_kernel_spmd(nc, in_maps, core_ids=list(range(N_CORES)))
    outs = [res.results[k]["out"].reshape(BSH) for k in range(N_CORES)]
    return np.concatenate(outs).astype(np.float32)
